# revision 1
# baseline (speedup 1.0000x reference)
"""Causal self-attention (B=4, T=2048, C=2048, H=16, HD=128) on 8 trn2 cores.

Sharding: core c handles batch b = c//2 and heads (c%2)*8 .. +8.
  - QKV projection column-sharded by head, attention head-sharded,
    c_proj row-sharded; the pair partial sums are combined on host.
Matmuls run in float32r (TF32-like, ~1.3e-4 rel err, 4x fp32 speed).

Self-contained: hardcodes shapes; builds one SPMD Bass program and runs
it on cores 0-7 via run_bass_kernel_spmd.
"""
import math

import numpy as np

import concourse.bass as bass
import concourse.mybir as mybir
import concourse.tile as tile
from concourse.bass_utils import run_bass_kernel_spmd

F32 = mybir.dt.float32
F32R = mybir.dt.float32r
AF = mybir.ActivationFunctionType
ALU = mybir.AluOpType

# problem dims
B, T, C, H = 4, 2048, 2048, 16
HD = 128
NCORES = 8
NH = H // 2          # heads per core
MCH = 512            # matmul moving-operand chunk (max for 4-byte dtypes)

_ctr = [0]


def _legalize_waits(nc, max_waits=1):
    """This walrus build rejects >1 sync wait per instruction. Hoist extra
    waits onto same-engine NoOps inserted directly before the instruction."""
    n_split = 0
    for f in nc.m.functions:
        for blk in f.blocks:
            newil = []
            changed = False
            for inst in blk.instructions:
                si = inst.sync_info
                if si is not None and si.on_wait and len(si.on_wait) > max_waits:
                    waits = list(si.on_wait)
                    for w in waits[:-max_waits]:
                        _ctr[0] += 1
                        nop = mybir.InstNoOp(name=f"I-waitfix-{_ctr[0]}")
                        nop.engine = inst.engine
                        nop.sync_info = mybir.SyncInfo(on_wait=[w], on_update=[])
                        newil.append(nop)
                    inst.sync_info = mybir.SyncInfo(
                        on_wait=waits[-max_waits:], on_update=list(si.on_update)
                    )
                    changed = True
                    n_split += 1
                newil.append(inst)
            if changed:
                blk.instructions = newil
    return n_split


def build_program(T=T, C=C, NH=NH, use_bqkv=False, qtile=512, legalize=True):
    """One core's program: full pipeline for (1 batch, NH heads)."""
    CB = C // 128          # contraction blocks
    TBn = T // 128         # token blocks
    QTILE = min(qtile, T)  # flash tq tile (>=256 for f32r full rate)
    NQT = T // QTILE
    JMAX = QTILE // 128
    DV = NH * 128          # v/proj-shard width
    inv_sqrt_hd = 1.0 / math.sqrt(HD)

    nc = bass.Bass()
    xt_d = nc.dram_tensor("xt", [C, T], F32R, kind="ExternalInput")
    wqk_d = nc.dram_tensor("wqk", [2, NH, 128, C], F32R, kind="ExternalInput")
    wv_d = nc.dram_tensor("wv", [CB, 128, DV], F32R, kind="ExternalInput")
    wp_d = nc.dram_tensor("wp", [NH, 128, C], F32R, kind="ExternalInput")
    cos2_d = nc.dram_tensor("cos2", [128, T], F32, kind="ExternalInput")
    sin2s_d = nc.dram_tensor("sin2s", [128, T], F32, kind="ExternalInput")
    mask_d = nc.dram_tensor("maskbig", [128, 2 * QTILE - 128], F32R, kind="ExternalInput")
    ones_d = nc.dram_tensor("ones128", [128, 128], F32R, kind="ExternalInput")
    if use_bqkv:
        bqk_d = nc.dram_tensor("bqk", [128, 2 * NH], F32, kind="ExternalInput")
        onecol_d = nc.dram_tensor("onecol", [1, 128], F32R, kind="ExternalInput")
        bv_d = nc.dram_tensor("bv", [1, DV], F32R, kind="ExternalInput")
    out_d = nc.dram_tensor("out_partial", [T, C], F32, kind="ExternalOutput")

    q_sp = nc.dram_tensor("q_spill", [NH, 128, T], F32R)
    k_sp = nc.dram_tensor("k_spill", [NH, 128, T], F32R)
    v_sp = nc.dram_tensor("v_spill", [TBn, 128, DV], F32R)

    with tile.TileContext(nc) as tc:
        with (
            tc.tile_pool(name="xpool", bufs=1) as xpool,
            tc.tile_pool(name="cpool", bufs=1) as cpool,
        ):
            xts = []
            for cb in range(CB):
                xt = xpool.tile([128, T], F32R, tag=f"x{cb}")
                nc.sync.dma_start(out=xt[:], in_=xt_d[cb * 128:(cb + 1) * 128, :])
                xts.append(xt)
            if use_bqkv:
                bqk = cpool.tile([128, 2 * NH], F32)
                nc.sync.dma_start(out=bqk[:], in_=bqk_d[:])
                onecol = cpool.tile([1, 128], F32R)
                nc.sync.dma_start(out=onecol[:], in_=onecol_d[:])
                bv = cpool.tile([1, DV], F32R)
                nc.sync.dma_start(out=bv[:], in_=bv_d[:])

            # ---------------- Phase A1: V = x @ Wv (t-major) ----------------
            nhalf = max(1, DV // 512)
            hw = DV // nhalf  # half width (<=512)
            for half in range(nhalf):
                c0 = half * hw
                with (
                    tc.tile_pool(name="wvpool", bufs=1) as wvpool,
                    tc.tile_pool(name="vepool", bufs=2) as vepool,
                    tc.tile_pool(name="psv", bufs=2, space="PSUM") as psvp,
                ):
                    wvts = []
                    for cb in range(CB):
                        wvt = wvpool.tile([128, hw], F32R, tag=f"wv{cb}")
                        nc.sync.dma_start(out=wvt[:], in_=wv_d[cb, :, c0:c0 + hw])
                        wvts.append(wvt)
                    for tb in range(TBn):
                        psv = psvp.tile([128, hw], F32, tag="psv")
                        for cb in range(CB):
                            nc.tensor.matmul(
                                psv[:],
                                xts[cb][:, tb * 128:(tb + 1) * 128],
                                wvts[cb][:],
                                start=(cb == 0),
                                stop=(cb == CB - 1 and not use_bqkv),
                            )
                        if use_bqkv:
                            nc.tensor.matmul(psv[:], onecol[:], bv[:, c0:c0 + hw],
                                             start=False, stop=True)
                        vsb = vepool.tile([128, hw], F32R, tag="vsb")
                        nc.scalar.copy(out=vsb[:], in_=psv[:])
                        nc.sync.dma_start(out=v_sp[tb, :, c0:c0 + hw], in_=vsb[:])

            # ------------- Phase A2: q^T, k^T per head + RoPE -------------
            with (
                tc.tile_pool(name="cspool", bufs=1) as cspool,
                tc.tile_pool(name="qepool", bufs=2) as qepool,
                tc.tile_pool(name="wqpool", bufs=2) as wqpool,
                tc.tile_pool(name="psq", bufs=2, space="PSUM") as psqp,
            ):
                cos2 = cspool.tile([128, T], F32)
                nc.sync.dma_start(out=cos2[:], in_=cos2_d[:])
                sin2s = cspool.tile([128, T], F32)
                nc.sync.dma_start(out=sin2s[:], in_=sin2s_d[:])
                for s in range(2):
                    spill = q_sp if s == 0 else k_sp
                    for h in range(NH):
                        wq = wqpool.tile([128, C], F32R, tag="wq")
                        nc.sync.dma_start(out=wq[:], in_=wqk_d[s, h])
                        ps = psqp.tile([128, T], F32, tag="psq")
                        for t0 in range(0, T, MCH):
                            for cb in range(CB):
                                nc.tensor.matmul(
                                    ps[:, t0:t0 + MCH],
                                    wq[:, cb * 128:(cb + 1) * 128],
                                    xts[cb][:, t0:t0 + MCH],
                                    start=(cb == 0),
                                    stop=(cb == CB - 1),
                                )
                        hw2 = T // 2
                        for half in range(2):
                            hs = slice(half * hw2, (half + 1) * hw2)
                            qb = qepool.tile([128, hw2], F32, tag="qb")
                            if use_bqkv:
                                nc.vector.tensor_scalar(
                                    qb[:], ps[:, hs], bqk[:, s * NH + h:s * NH + h + 1],
                                    None, ALU.add)
                            else:
                                nc.scalar.copy(out=qb[:], in_=ps[:, hs])
                            qrot = qepool.tile([128, hw2], F32, tag="qrot")
                            nc.sync.dma_start(out=qrot[0:64, :], in_=qb[64:128, :])
                            nc.sync.dma_start(out=qrot[64:128, :], in_=qb[0:64, :])
                            nc.vector.tensor_mul(qb[:], qb[:], cos2[:, hs])
                            nc.vector.tensor_mul(qrot[:], qrot[:], sin2s[:, hs])
                            qr = qepool.tile([128, hw2], F32R, tag="qr")
                            nc.vector.tensor_add(qr[:], qb[:], qrot[:])
                            nc.sync.dma_start(out=spill[h, :, hs], in_=qr[:])

        # ---------------- Phase B: causal flash attention ----------------
        # O^T stays resident in SBUF across Phase B -> C (no DRAM round-trip)
        opersist_cm = tc.tile_pool(name="opersist", bufs=1)
        opersist = opersist_cm.__enter__()
        ohs = [opersist.tile([128, T], F32R, name=f"oh{hd}", tag=f"oh{hd}") for hd in range(NH)]
        with (
            tc.tile_pool(name="fpool", bufs=2) as fpool,
            tc.tile_pool(name="bcpool", bufs=1) as bcpool,
            tc.tile_pool(name="ppool", bufs=6) as ppool,
            tc.tile_pool(name="ropool", bufs=2) as ropool,
            tc.tile_pool(name="psS", bufs=4, space="PSUM") as psSp,
            tc.tile_pool(name="psO", bufs=2, space="PSUM") as psOp,
            tc.tile_pool(name="psR", bufs=2, space="PSUM") as psRp,
        ):
            maskt = bcpool.tile([128, 2 * QTILE - 128], F32R)
            nc.sync.dma_start(out=maskt[:], in_=mask_d[:])
            ones = bcpool.tile([128, 128], F32R)
            nc.sync.dma_start(out=ones[:], in_=ones_d[:])
            for h in range(NH):
                qr = fpool.tile([128, T], F32R, tag="qrh")
                nc.sync.dma_start(out=qr[:], in_=q_sp[h])
                kr = fpool.tile([128, T], F32R, tag="krh")
                nc.sync.dma_start(out=kr[:], in_=k_sp[h])
                vh = fpool.tile([128, TBn, 128], F32R, tag="vh")
                nc.sync.dma_start(
                    out=vh[:],
                    in_=v_sp[:, :, h * 128:(h + 1) * 128].transpose([1, 0, 2]))
                for qt in range(NQT):
                    ntk = (qt + 1) * JMAX
                    tqs = slice(qt * QTILE, (qt + 1) * QTILE)
                    psO = psOp.tile([128, QTILE], F32, tag="psO")
                    psR = psRp.tile([128, QTILE], F32, tag="psR")
                    for tkb in range(ntk):
                        psS = psSp.tile([128, QTILE], F32, tag="psS")
                        nc.tensor.matmul(
                            psS[:], kr[:, tkb * 128:(tkb + 1) * 128], qr[:, tqs],
                            start=True, stop=True)
                        pt = ppool.tile([128, QTILE], F32R, tag="pt")
                        nc.scalar.activation(pt[:], psS[:], AF.Exp, scale=inv_sqrt_hd)
                        j = tkb - qt * JMAX
                        if j >= 0:
                            m0 = (JMAX - 1 - j) * 128
                            nc.vector.tensor_mul(pt[:], pt[:], maskt[:, m0:m0 + QTILE])
                        nc.tensor.matmul(psO[:], vh[:, tkb, :], pt[:],
                                         start=(tkb == 0), stop=(tkb == ntk - 1))
                        nc.tensor.matmul(psR[:], ones[:], pt[:],
                                         start=(tkb == 0), stop=(tkb == ntk - 1))
                    rec = ropool.tile([128, QTILE], F32, tag="rec")
                    nc.vector.reciprocal(rec[:], psR[:])
                    nc.vector.tensor_mul(ohs[h][:, tqs], psO[:], rec[:])

        # ---------------- Phase C: out_partial = O @ Wp_shard ----------------
        with (
            tc.tile_pool(name="wppool", bufs=1) as wppool,
            tc.tile_pool(name="oepool", bufs=2) as oepool,
            tc.tile_pool(name="psP", bufs=2, space="PSUM") as psPp,
        ):
            wps = []
            for hd in range(NH):
                wpt = wppool.tile([128, C], F32R, tag=f"wp{hd}")
                nc.sync.dma_start(out=wpt[:], in_=wp_d[hd])
                wps.append(wpt)
            for tb in range(TBn):
                psP = psPp.tile([128, C], F32, tag="psP")
                for c0 in range(0, C, MCH):
                    for hd in range(NH):
                        nc.tensor.matmul(
                            psP[:, c0:c0 + MCH],
                            ohs[hd][:, tb * 128:(tb + 1) * 128],
                            wps[hd][:, c0:c0 + MCH],
                            start=(hd == 0), stop=(hd == NH - 1))
                outsb = oepool.tile([128, C], F32, tag="outsb")
                nc.scalar.copy(out=outsb[:], in_=psP[:])
                nc.sync.dma_start(out=out_d[tb * 128:(tb + 1) * 128, :], in_=outsb[:])
        opersist_cm.__exit__(None, None, None)

    if legalize:
        _legalize_waits(nc)
    return nc


# ---------------------------------------------------------------- host side

_PERM = np.concatenate([np.arange(0, HD, 2), np.arange(1, HD, 2)])  # de-interleave


def shard_core(core, x, freqs_cos, freqs_sin, Wqkv, bqkv, Wproj,
               T=T, C=C, NH=NH, qtile=512, use_bqkv=False):
    """Build the in_map for one core."""
    CB = C // 128
    DV = NH * 128
    QTILE = min(qtile, T)
    b = core // 2
    hb = (core % 2) * NH

    xt = np.ascontiguousarray(x[b].T).astype(np.float32)

    # [2, NH, 128] column indices (q/k, de-interleaved within each head)
    cols = (np.arange(2)[:, None, None] * C
            + (hb + np.arange(NH))[None, :, None] * HD + _PERM[None, None, :])
    wqk = Wqkv[:, cols]                              # [C, 2, NH, 128]
    wqk = np.ascontiguousarray(
        wqk.reshape(CB, 128, 2, NH, 128).transpose(2, 3, 1, 0, 4)
        .reshape(2, NH, 128, C))

    wv = np.ascontiguousarray(
        Wqkv[:, 2 * C + hb * HD: 2 * C + (hb + NH) * HD].reshape(CB, 128, DV))
    wp = np.ascontiguousarray(
        Wproj[hb * HD:(hb + NH) * HD, :].reshape(NH, 128, C))

    cos2 = np.concatenate([freqs_cos.T, freqs_cos.T], 0).astype(np.float32)
    cos2 = np.ascontiguousarray(cos2)                # [128, T]
    sin2s = np.concatenate([-freqs_sin.T, freqs_sin.T], 0).astype(np.float32)
    sin2s = np.ascontiguousarray(sin2s)

    u = np.arange(2 * QTILE - 128)[None, :]
    p = np.arange(128)[:, None]
    maskbig = (p <= u - (QTILE - 128)).astype(np.float32)

    im = {
        "xt": xt, "wqk": wqk, "wv": wv, "wp": wp,
        "cos2": cos2, "sin2s": sin2s, "maskbig": maskbig,
        "ones128": np.ones((128, 128), np.float32),
    }
    if use_bqkv:
        bqk = np.empty((128, 2 * NH), np.float32)
        for s in range(2):
            for h in range(NH):
                bqk[:, s * NH + h] = bqkv[s * C + (hb + h) * HD + _PERM]
        im["bqk"] = bqk
        im["onecol"] = np.ones((1, 128), np.float32)
        im["bv"] = np.ascontiguousarray(
            bqkv[2 * C + hb * HD: 2 * C + (hb + NH) * HD][None, :])
    return im


_CACHE = {}


def _get_program(use_bqkv):
    key = use_bqkv
    if key not in _CACHE:
        _CACHE[key] = build_program(use_bqkv=use_bqkv)
    return _CACHE[key]


def kernel(x, freqs_cos, freqs_sin, Wqkv, bqkv, Wproj, bproj):
    x = np.asarray(x, np.float32)
    freqs_cos = np.asarray(freqs_cos, np.float32)
    freqs_sin = np.asarray(freqs_sin, np.float32)
    Wqkv = np.asarray(Wqkv, np.float32)
    bqkv = np.asarray(bqkv, np.float32)
    Wproj = np.asarray(Wproj, np.float32)
    bproj = np.asarray(bproj, np.float32)

    use_bqkv = bool(np.any(bqkv != 0))
    nc = _get_program(use_bqkv)
    in_maps = [
        shard_core(c, x, freqs_cos, freqs_sin, Wqkv, bqkv, Wproj,
                   use_bqkv=use_bqkv)
        for c in range(NCORES)
    ]
    try:
        res = run_bass_kernel_spmd(nc, in_maps, list(range(NCORES))).results
    except Exception:
        # transient device faults have been observed; retry once
        res = run_bass_kernel_spmd(nc, in_maps, list(range(NCORES))).results

    out = np.empty((B, T, C), np.float32)
    for b in range(B):
        out[b] = res[2 * b]["out_partial"] + res[2 * b + 1]["out_partial"]
    out += bproj[None, None, :]
    return out



# revision 18
# speedup vs baseline: 1.1692x; 1.1692x over previous
"""Causal self-attention (B=4, T=2048, C=2048, H=16, HD=128) on 8 trn2 cores.

Sharding: core c handles batch b = c//2 and heads (c%2)*8 .. +8.
  - QKV projection column-sharded by head, attention head-sharded,
    c_proj row-sharded; the pair partial sums are combined on host.

v3: all matmul operands bf16 (same PE rate as f32r in the cost model,
half the DMA/SBUF), fully SBUF-resident pipeline (no q/k/v DRAM spill),
fused per-head schedule: project head h+1 while attention for head h
runs on the PE; RoPE on DVE overlaps attention; flash tiles of 256
queries to trim the causal diagonal.

Self-contained: hardcodes shapes; builds one SPMD Bass program and runs
it on cores 0-7 via run_bass_kernel_spmd.
"""
import math

import ml_dtypes
import numpy as np

import concourse.bass as bass
import concourse.mybir as mybir
import concourse.tile as tile
from concourse.bass_utils import run_bass_kernel_spmd

F32 = mybir.dt.float32
BF16 = mybir.dt.bfloat16
AF = mybir.ActivationFunctionType
ALU = mybir.AluOpType
NPBF = ml_dtypes.bfloat16

# problem dims
B, T, C, H = 4, 2048, 2048, 16
HD = 128
NCORES = 8
NH = H // 2          # heads per core

_ctr = [0]


def _legalize_waits(nc, max_waits=1):
    """This walrus build rejects >1 sync wait per instruction. Hoist extra
    waits onto same-engine NoOps inserted directly before the instruction."""
    n_split = 0
    for f in nc.m.functions:
        for blk in f.blocks:
            newil = []
            changed = False
            for inst in blk.instructions:
                si = inst.sync_info
                if si is not None and si.on_wait and len(si.on_wait) > max_waits:
                    waits = list(si.on_wait)
                    for w in waits[:-max_waits]:
                        _ctr[0] += 1
                        nop = mybir.InstNoOp(name=f"I-waitfix-{_ctr[0]}")
                        nop.engine = inst.engine
                        nop.sync_info = mybir.SyncInfo(on_wait=[w], on_update=[])
                        newil.append(nop)
                    inst.sync_info = mybir.SyncInfo(
                        on_wait=waits[-max_waits:], on_update=list(si.on_update)
                    )
                    changed = True
                    n_split += 1
                newil.append(inst)
            if changed:
                blk.instructions = newil
    return n_split


def build_program(T=T, C=C, NH=NH, use_bqkv=False, qtile=256, legalize=True):
    """One core's program: full pipeline for (1 batch, NH heads)."""
    CB = C // 128          # contraction blocks
    TBn = T // 128         # token blocks
    QTILE = min(qtile, T)  # flash query-tile
    NQT = T // QTILE
    JMAX = QTILE // 128
    DV = NH * 128          # v/proj-shard width
    NG = max(1, DV // 512)  # v-projection head groups (512 cols each)
    GW = DV // NG           # group width
    GH = NH // NG           # heads per group
    TCH = 512               # xt column chunk / qk psq chunk
    NTC = T // TCH
    inv_sqrt_hd = 1.0 / math.sqrt(HD)

    nc = bass.Bass()
    xt_d = nc.dram_tensor("xt", [CB, 128, T], BF16, kind="ExternalInput")
    wqk_d = nc.dram_tensor("wqk", [2, NH, 128, C], BF16, kind="ExternalInput")
    wv_d = nc.dram_tensor("wv", [CB, 128, DV], BF16, kind="ExternalInput")
    wp_d = nc.dram_tensor("wp", [NH, 128, C], BF16, kind="ExternalInput")
    cos2_d = nc.dram_tensor("cos2", [128, T], BF16, kind="ExternalInput")
    sin2s_d = nc.dram_tensor("sin2s", [128, T], BF16, kind="ExternalInput")
    mask_d = nc.dram_tensor("maskbig", [128, 2 * QTILE - 128], BF16, kind="ExternalInput")
    ones_d = nc.dram_tensor("ones128", [128, 128], BF16, kind="ExternalInput")
    if use_bqkv:
        # [128, 2*NH] per-partition q/k bias columns; V bias via rank-1 matmul
        bqk_d = nc.dram_tensor("bqk", [128, 2 * NH], F32, kind="ExternalInput")
        onecol_d = nc.dram_tensor("onecol", [1, 128], BF16, kind="ExternalInput")
        bv_d = nc.dram_tensor("bv", [1, DV], BF16, kind="ExternalInput")
    out_d = nc.dram_tensor("out_partial", [T, C], BF16, kind="ExternalOutput")

    with tile.TileContext(nc) as tc:
        cms = {}

        def openpool(name, **kw):
            cm = tc.tile_pool(name=name, bufs=1, **kw)
            cms[name] = cm
            return cm.__enter__()

        def close(*names):
            for n in names:
                cms.pop(n).__exit__(None, None, None)

        # ---- pools + tiles up front, in per-side stack order.
        # left SBUF stack (live to the end): cpool..ropool;
        # right stack: xpool, later replaced by wppool.
        cpool = openpool("cpool")
        cos2 = cpool.tile([128, T], BF16, name="cos2")
        sin2s = cpool.tile([128, T], BF16, name="sin2s")
        maskt = cpool.tile([128, 2 * QTILE - 128], BF16, name="maskt")
        ones = cpool.tile([128, 128], BF16, name="ones")
        if use_bqkv:
            bqk = cpool.tile([128, 2 * NH], F32, name="bqk")
            onecol = cpool.tile([1, 128], BF16, name="onecol")
            bv = cpool.tile([1, DV], BF16, name="bv")

        ohpool = openpool("ohpool")
        ohs = [ohpool.tile([128, T], BF16, name=f"oh{h}", tag=f"oh{h}")
               for h in range(NH)]

        wqpool = openpool("wqpool")
        wq_r = [(wqpool.tile([128, C], BF16, name=f"wq{i}", tag=f"wq{i}"),
                 wqpool.tile([128, C], BF16, name=f"wk{i}", tag=f"wk{i}"))
                for i in range(2)]

        wvpool = openpool("wvpool")
        wvg_t = wvpool.tile([128, CB, GW], BF16, name="wvg")

        vpool = openpool("vpool")
        vgs = [[vpool.tile([128, GW], BF16, name=f"vg{g}_{tb}", tag=f"vg{g}_{tb}")
                for tb in range(TBn)] for g in range(NG)]

        qkpool = openpool("qkpool")
        qk_r = [(qkpool.tile([128, T], BF16, name=f"qr{i}", tag=f"qr{i}"),
                 qkpool.tile([128, T], BF16, name=f"kr{i}", tag=f"kr{i}"))
                for i in range(3)]

        ppool = openpool("ppool")
        qb_r = [ppool.tile([128, TCH], BF16, name=f"qb{i}", tag=f"qb{i}")
                for i in range(4)]
        qrot_r = [ppool.tile([128, TCH], BF16, name=f"qrot{i}", tag=f"qrot{i}")
                  for i in range(4)]
        pt_r = [ppool.tile([128, QTILE], BF16, name=f"pt{i}", tag=f"pt{i}")
                for i in range(5)]

        ropool = openpool("ropool")
        rec_r = [ropool.tile([128, QTILE], F32, name=f"rec{i}", tag=f"rec{i}")
                 for i in range(2)]
        racc_r = [ropool.tile([128, QTILE], BF16, name=f"racc{i}", tag=f"racc{i}")
                  for i in range(2)]

        xpool = openpool("xpool", side="right")
        xts = [xpool.tile([128, T], BF16, name=f"x{cb}", tag=f"x{cb}")
               for cb in range(CB)]

        # PSUM: prologue uses psq(2)+psv(2); psv then closes and the
        # attention pools take its banks -> psq2+psS2+psO2+psR2 = 8 banks.
        psqp = openpool("psq", space="PSUM")
        psq_r = [psqp.tile([128, TCH], F32, name=f"psq{i}", tag=f"psq{i}")
                 for i in range(2)]
        psvp = openpool("psv", space="PSUM")
        psv_r = [psvp.tile([128, GW], F32, name=f"psv{i}", tag=f"psv{i}")
                 for i in range(2)]
        psS_r, psO_r, psR_r = [], [], []

        ctr = {"psv": 0, "psq": 0, "psS": 0, "psO": 0, "qb": 0, "pt": 0,
               "rec": 0, "racc": 0}

        def ring(rs, key):
            t = rs[ctr[key] % len(rs)]
            ctr[key] += 1
            return t

        # ---------------- DMA preloads (issue order = queue order) ----------
        def load_wq(h):
            wq, wk = wq_r[h % 2]
            nc.sync.dma_start(out=wq[:], in_=wqk_d[0, h])
            nc.sync.dma_start(out=wk[:], in_=wqk_d[1, h])

        def load_wvg(g):
            for cb in range(CB):
                nc.sync.dma_start(out=wvg_t[:, cb, :],
                                  in_=wv_d[cb, :, g * GW:(g + 1) * GW])

        def load_xt_chunk(tc_):
            for cb in range(CB):
                nc.sync.dma_start(out=xts[cb][:, tc_ * TCH:(tc_ + 1) * TCH],
                                  in_=xt_d[cb, :, tc_ * TCH:(tc_ + 1) * TCH])

        nc.sync.dma_start(out=wq_r[0][0][:], in_=wqk_d[0, 0])
        load_xt_chunk(0)
        nc.sync.dma_start(out=wq_r[0][1][:], in_=wqk_d[1, 0])
        load_wvg(0)
        load_xt_chunk(1)
        nc.sync.dma_start(out=cos2[:], in_=cos2_d[:])
        nc.sync.dma_start(out=sin2s[:], in_=sin2s_d[:])
        for tc_ in range(2, NTC):
            load_xt_chunk(tc_)
        nc.sync.dma_start(out=maskt[:], in_=mask_d[:])
        nc.sync.dma_start(out=ones[:], in_=ones_d[:])
        load_wq(1)
        if use_bqkv:
            nc.sync.dma_start(out=bqk[:], in_=bqk_d[:])
            nc.sync.dma_start(out=onecol[:], in_=onecol_d[:])
            nc.sync.dma_start(out=bv[:], in_=bv_d[:])

        # ---------------- building blocks ----------------
        def vproj_group(g, tb0, tb1):
            """V columns for head group g, token blocks [tb0, tb1)."""
            for tb in range(tb0, tb1):
                psv = ring(psv_r, "psv")
                for cb in range(CB):
                    nc.tensor.matmul(
                        psv[:], xts[cb][:, tb * 128:(tb + 1) * 128], wvg_t[:, cb, :],
                        start=(cb == 0), stop=(cb == CB - 1 and not use_bqkv))
                if use_bqkv:
                    nc.tensor.matmul(psv[:], onecol[:], bv[:, g * GW:(g + 1) * GW],
                                     start=False, stop=True)
                nc.scalar.copy(out=vgs[g][tb][:], in_=psv[:])

        def rope_tail(h, s, tc_, ps):
            """PSUM chunk -> RoPE -> qr/kr slice (ACT+DMA+DVE, no PE work)."""
            dst = qk_r[h % 3][s]
            ts = slice(tc_ * TCH, (tc_ + 1) * TCH)
            qb = ring(qb_r, "qb")
            qrot = qrot_r[(ctr["qb"] - 1) % len(qrot_r)]
            if use_bqkv:
                nc.vector.tensor_scalar(
                    qb[:], ps[:], bqk[:, s * NH + h:s * NH + h + 1], None, ALU.add)
            else:
                nc.scalar.copy(out=qb[:], in_=ps[:])
            # partition-half swap on the idle gpsimd DMA queue (keeps the
            # SP preload and Activation queues free)
            nc.gpsimd.dma_start(out=qrot[0:64, :], in_=qb[64:128, :])
            nc.gpsimd.dma_start(out=qrot[64:128, :], in_=qb[0:64, :])
            nc.vector.tensor_mul(qb[:], qb[:], cos2[:, ts])
            nc.vector.tensor_mul(qrot[:], qrot[:], sin2s[:, ts])
            nc.vector.tensor_add(dst[:, ts], qb[:], qrot[:])

        def qkproj_chunk(h, s, tc_):
            """psq for (head h, q/k s), token chunk tc_, then RoPE."""
            w = wq_r[h % 2][s]
            ts = slice(tc_ * TCH, (tc_ + 1) * TCH)
            ps = ring(psq_r, "psq")
            for cb in range(CB):
                nc.tensor.matmul(ps[:], w[:, cb * 128:(cb + 1) * 128],
                                 xts[cb][:, ts], start=(cb == 0), stop=(cb == CB - 1))
            rope_tail(h, s, tc_, ps)

        def qkproj_head(h):
            for tc_ in range(NTC):
                qkproj_chunk(h, 0, tc_)
                qkproj_chunk(h, 1, tc_)

        def proj_gen(h):
            """Generator form of qkproj_head: yields after each PE matmul so
            projection work can be woven into an attention stream."""
            for tc_ in range(NTC):
                for s in (0, 1):
                    w = wq_r[h % 2][s]
                    ts = slice(tc_ * TCH, (tc_ + 1) * TCH)
                    ps = ring(psq_r, "psq")
                    for cb in range(CB):
                        nc.tensor.matmul(
                            ps[:], w[:, cb * 128:(cb + 1) * 128], xts[cb][:, ts],
                            start=(cb == 0), stop=(cb == CB - 1))
                        yield
                    rope_tail(h, s, tc_, ps)

        pending = []   # deferred per-qt R matmuls + norms, shared across heads

        def attn_gen(h, accum_R=False):
            """Generator: one flash-attention head; yields after each key
            block.  S matmuls run LOOKAHEAD blocks ahead of their O/R
            consumers so the exp result is ready before the PE needs it --
            the PE must run back-to-back to hold its top p-state.

            accum_R=True replaces the per-block ones-matmul with bf16 DVE
            adds into racc and a single deferred ones-matmul per query tile
            (emitted early in the NEXT tile's stream, with the norm)."""
            LOOKAHEAD = 2
            qr, kr = qk_r[h % 3]
            g, hi = divmod(h, GH)
            vg = vgs[g]

            def norm_tail(psO, psR, tqs):
                rec = ring(rec_r, "rec")
                nc.vector.reciprocal(rec[:], psR[:])
                nc.vector.tensor_mul(ohs[h][:, tqs], psO[:], rec[:])

            for qt in range(NQT):
                ntk = (qt + 1) * JMAX
                tqs = slice(qt * QTILE, (qt + 1) * QTILE)
                psO = ring(psO_r, "psO")
                psR = psR_r[(ctr["psO"] - 1) % len(psR_r)]
                racc = ring(racc_r, "racc") if accum_R else None
                pts = {}

                def emit_S(tkb, ntk=ntk, qt=qt, pts=pts, racc=racc):
                    psS = ring(psS_r, "psS")
                    nc.tensor.matmul(
                        psS[:], kr[:, tkb * 128:(tkb + 1) * 128], qr[:, tqs],
                        start=True, stop=True)
                    pt = ring(pt_r, "pt")
                    nc.scalar.activation(pt[:], psS[:], AF.Exp, scale=inv_sqrt_hd)
                    j = tkb - qt * JMAX
                    if j >= 0:
                        m0 = (JMAX - 1 - j) * 128
                        nc.vector.tensor_mul(pt[:], pt[:], maskt[:, m0:m0 + QTILE])
                    pts[tkb] = pt
                    if racc is not None and tkb >= 1:
                        a = pts[0] if tkb == 1 else racc
                        nc.vector.tensor_add(racc[:], a[:], pt[:])

                def emit_OR(tkb, ntk=ntk, pts=pts, psO=psO, psR=psR):
                    pt = pts[tkb]
                    nc.tensor.matmul(psO[:], vg[tkb][:, hi * 128:(hi + 1) * 128],
                                     pt[:], start=(tkb == 0), stop=(tkb == ntk - 1))
                    if not accum_R:
                        nc.tensor.matmul(psR[:], ones[:], pt[:],
                                         start=(tkb == 0), stop=(tkb == ntk - 1))

                for tkb in range(ntk):
                    emit_S(tkb)
                    if tkb >= LOOKAHEAD:
                        emit_OR(tkb - LOOKAHEAD)
                    if pending and (fn := pending.pop(0)) is not None:
                        fn()
                    yield
                for tkb in range(max(0, ntk - LOOKAHEAD), ntk):
                    emit_OR(tkb)
                    if pending and (fn := pending.pop(0)) is not None:
                        fn()
                    yield
                if accum_R:
                    def r_matmul(racc=racc, psR=psR):
                        nc.tensor.matmul(psR[:], ones[:], racc[:],
                                         start=True, stop=True)
                    pending.extend([None, None, r_matmul,
                                    (lambda psO=psO, psR=psR, tqs=tqs:
                                     norm_tail(psO, psR, tqs))])
                else:
                    norm_tail(psO, psR, tqs)

        def run_attn(h, filler=None):
            """Emit attention head h, weaving in up to 2 filler matmuls per
            key block to keep the PE fed across the exp latency chain."""
            for _ in attn_gen(h, accum_R=True):
                if filler is not None:
                    for _ in range(2):
                        if next(filler, "done") == "done":
                            filler = None
                            break
            if filler is not None:
                for _ in filler:
                    pass

        def run_attn_pair(h0, h1):
            """Interleave two attention heads block-by-block; each absorbs
            the other's softmax latency."""
            import itertools
            for _ in itertools.zip_longest(attn_gen(h0), attn_gen(h1)):
                pass

        # ---------------- prologue: v (all groups) + heads 0,1 projections --
        tbpg = TBn // NTC
        for tc_ in range(NTC):
            qkproj_chunk(0, 0, tc_)
            qkproj_chunk(0, 1, tc_)
            vproj_group(0, tc_ * tbpg, (tc_ + 1) * tbpg)
        if NH > 2:
            load_wq(2)
        if NG > 1:
            load_wvg(1)
            for tc_ in range(NTC):
                qkproj_chunk(1, 0, tc_)
                qkproj_chunk(1, 1, tc_)
                vproj_group(1, tc_ * tbpg, (tc_ + 1) * tbpg)
        else:
            qkproj_head(1)

        # swap psv banks for the attention accumulators
        close("psv")
        psSp = openpool("psS", space="PSUM")
        psS_r.extend(psSp.tile([128, QTILE], F32, name=f"psS{i}", tag=f"psS{i}")
                     for i in range(2))
        psOp = openpool("psO", space="PSUM")
        psO_r.extend(psOp.tile([128, QTILE], F32, name=f"psO{i}", tag=f"psO{i}")
                     for i in range(2))
        psRp = openpool("psR", space="PSUM")
        psR_r.extend(psRp.tile([128, QTILE], F32, name=f"psR{i}", tag=f"psR{i}")
                     for i in range(2))

        # ---- steady state: attn(h) with head h+2's projection woven in ----
        for h in range(max(0, NH - 2)):
            if h + 3 < NH:
                load_wq(h + 3)
            run_attn(h, proj_gen(h + 2))

        # x is done (last read: head NH-1's projection); its SBUF hosts the
        # c_proj weights, whose DMA overlaps the last two attention heads
        close("xpool")
        wppool = tc.tile_pool(name="wppool", bufs=1, side="right")
        cms["wppool"] = wppool
        wppool_p = wppool.__enter__()
        wps = []
        for hd in range(NH):
            wpt = wppool_p.tile([128, C], BF16, name=f"wp{hd}", tag=f"wp{hd}")
            nc.sync.dma_start(out=wpt[:], in_=wp_d[hd])
            wps.append(wpt)

        # last two heads: no projection filler left; interleave them
        run_attn_pair(NH - 2, NH - 1)
        for fn in pending:
            if fn is not None:
                fn()

        # release the PSUM pools so phase C gets all 8 banks
        close("psR", "psO", "psS", "psq")

        # ---------------- Phase C: out_partial = O @ Wp_shard ----------------
        with (
            tc.tile_pool(name="oepool", bufs=2) as oepool,
            tc.tile_pool(name="psP", bufs=2, space="PSUM") as psPp,
        ):
            for tb in range(TBn):
                psP = psPp.tile([128, C], F32, tag="psP")
                outsb = oepool.tile([128, C], BF16, tag="outsb")
                rs = slice(tb * 128, (tb + 1) * 128)
                last = tb >= TBn - 2
                for c0 in range(0, C, 512):
                    for hd in range(NH):
                        nc.tensor.matmul(
                            psP[:, c0:c0 + 512],
                            ohs[hd][:, tb * 128:(tb + 1) * 128],
                            wps[hd][:, c0:c0 + 512],
                            start=(hd == 0), stop=(hd == NH - 1))
                    if last:
                        # drain chunk-by-chunk so the final copy+DMA is short
                        cq = slice(c0, c0 + 512)
                        nc.scalar.copy(out=outsb[:, cq], in_=psP[:, cq])
                        nc.sync.dma_start(out=out_d[rs, cq], in_=outsb[:, cq])
                if not last:
                    nc.scalar.copy(out=outsb[:], in_=psP[:])
                    nc.sync.dma_start(out=out_d[rs, :], in_=outsb[:])

        close("wppool", "ropool", "ppool", "qkpool", "vpool", "wvpool",
              "wqpool", "ohpool", "cpool")

    if legalize:
        _legalize_waits(nc)
    return nc


# ---------------------------------------------------------------- host side

_PERM = np.concatenate([np.arange(0, HD, 2), np.arange(1, HD, 2)])  # de-interleave


def shard_core(core, x, freqs_cos, freqs_sin, Wqkv, bqkv, Wproj,
               T=T, C=C, NH=NH, qtile=256, use_bqkv=False):
    """Build the in_map for one core."""
    CB = C // 128
    DV = NH * 128
    QTILE = min(qtile, T)
    b = core // 2
    hb = (core % 2) * NH

    xt = np.ascontiguousarray(x[b].T).reshape(CB, 128, T).astype(NPBF)

    # [2, NH, 128] column indices (q/k, de-interleaved within each head)
    cols = (np.arange(2)[:, None, None] * C
            + (hb + np.arange(NH))[None, :, None] * HD + _PERM[None, None, :])
    wqk = Wqkv[:, cols]                              # [C, 2, NH, 128]
    wqk = np.ascontiguousarray(
        wqk.reshape(CB, 128, 2, NH, 128).transpose(2, 3, 1, 0, 4)
        .reshape(2, NH, 128, C)).astype(NPBF)

    wv = np.ascontiguousarray(
        Wqkv[:, 2 * C + hb * HD: 2 * C + (hb + NH) * HD].reshape(CB, 128, DV)
    ).astype(NPBF)
    wp = np.ascontiguousarray(
        Wproj[hb * HD:(hb + NH) * HD, :].reshape(NH, 128, C)).astype(NPBF)

    cos2 = np.concatenate([freqs_cos.T, freqs_cos.T], 0)
    cos2 = np.ascontiguousarray(cos2).astype(NPBF)   # [128, T]
    sin2s = np.concatenate([-freqs_sin.T, freqs_sin.T], 0)
    sin2s = np.ascontiguousarray(sin2s).astype(NPBF)

    u = np.arange(2 * QTILE - 128)[None, :]
    p = np.arange(128)[:, None]
    maskbig = (p <= u - (QTILE - 128)).astype(NPBF)

    im = {
        "xt": xt, "wqk": wqk, "wv": wv, "wp": wp,
        "cos2": cos2, "sin2s": sin2s, "maskbig": maskbig,
        "ones128": np.ones((128, 128), NPBF),
    }
    if use_bqkv:
        bqk = np.empty((128, 2 * NH), np.float32)
        for s in range(2):
            for h in range(NH):
                bqk[:, s * NH + h] = bqkv[s * C + (hb + h) * HD + _PERM]
        im["bqk"] = bqk
        im["onecol"] = np.ones((1, 128), NPBF)
        im["bv"] = np.ascontiguousarray(
            bqkv[2 * C + hb * HD: 2 * C + (hb + NH) * HD][None, :]).astype(NPBF)
    return im


_CACHE = {}


def _get_program(use_bqkv):
    key = use_bqkv
    if key not in _CACHE:
        _CACHE[key] = build_program(use_bqkv=use_bqkv)
    return _CACHE[key]


def kernel(x, freqs_cos, freqs_sin, Wqkv, bqkv, Wproj, bproj):
    x = np.asarray(x, np.float32)
    freqs_cos = np.asarray(freqs_cos, np.float32)
    freqs_sin = np.asarray(freqs_sin, np.float32)
    Wqkv = np.asarray(Wqkv, np.float32)
    bqkv = np.asarray(bqkv, np.float32)
    Wproj = np.asarray(Wproj, np.float32)
    bproj = np.asarray(bproj, np.float32)

    use_bqkv = bool(np.any(bqkv != 0))
    nc = _get_program(use_bqkv)
    in_maps = [
        shard_core(c, x, freqs_cos, freqs_sin, Wqkv, bqkv, Wproj,
                   use_bqkv=use_bqkv)
        for c in range(NCORES)
    ]
    try:
        res = run_bass_kernel_spmd(nc, in_maps, list(range(NCORES))).results
    except Exception:
        # transient device faults have been observed; retry once
        res = run_bass_kernel_spmd(nc, in_maps, list(range(NCORES))).results

    out = np.empty((B, T, C), np.float32)
    for b in range(B):
        out[b] = (res[2 * b]["out_partial"].astype(np.float32)
                  + res[2 * b + 1]["out_partial"].astype(np.float32))
    out += bproj[None, None, :]
    return out


# revision 54
# speedup vs baseline: 1.2369x; 1.0580x over previous
"""Causal self-attention (B=4, T=2048, C=2048, H=16, HD=128) on 8 trn2 cores.

Sharding: core c handles batch b = c//2 and heads (c%2)*8 .. +8.
  - QKV projection column-sharded by head, attention head-sharded,
    c_proj row-sharded; the pair partial sums are combined on host.

v3: all matmul operands bf16 (same PE rate as f32r in the cost model,
half the DMA/SBUF), fully SBUF-resident pipeline (no q/k/v DRAM spill),
fused per-head schedule: project head h+1 while attention for head h
runs on the PE; RoPE on DVE overlaps attention; flash tiles of 256
queries to trim the causal diagonal.

Self-contained: hardcodes shapes; builds one SPMD Bass program and runs
it on cores 0-7 via run_bass_kernel_spmd.
"""
import math

import ml_dtypes
import numpy as np

import concourse.bass as bass
import concourse.bass_isa as bass_isa
import concourse.library_config as library_config
import concourse.mybir as mybir
import concourse.tile as tile
from concourse.bass_utils import run_bass_kernel_spmd

F32 = mybir.dt.float32
BF16 = mybir.dt.bfloat16
AF = mybir.ActivationFunctionType
ALU = mybir.AluOpType
NPBF = ml_dtypes.bfloat16

# problem dims
B, T, C, H = 4, 2048, 2048, 16
HD = 128
NCORES = 8
NH = H // 2          # heads per core

_ctr = [0]


def _legalize_waits(nc, max_waits=1):
    """This walrus build rejects >1 sync wait per instruction. Hoist extra
    waits onto same-engine NoOps inserted directly before the instruction."""
    n_split = 0
    for f in nc.m.functions:
        for blk in f.blocks:
            newil = []
            changed = False
            for inst in blk.instructions:
                si = inst.sync_info
                if si is not None and si.on_wait and len(si.on_wait) > max_waits:
                    waits = list(si.on_wait)
                    for w in waits[:-max_waits]:
                        _ctr[0] += 1
                        nop = mybir.InstNoOp(name=f"I-waitfix-{_ctr[0]}")
                        nop.engine = inst.engine
                        nop.sync_info = mybir.SyncInfo(on_wait=[w], on_update=[])
                        newil.append(nop)
                    inst.sync_info = mybir.SyncInfo(
                        on_wait=waits[-max_waits:], on_update=list(si.on_update)
                    )
                    changed = True
                    n_split += 1
                newil.append(inst)
            if changed:
                blk.instructions = newil
    return n_split


def build_program(T=T, C=C, NH=NH, use_bqkv=False, qtile=256, legalize=True):
    """One core's program: full pipeline for (1 batch, NH heads)."""
    CB = C // 128          # contraction blocks
    TBn = T // 128         # token blocks
    QTILE = min(qtile, T)  # flash query-tile
    NQT = T // QTILE
    JMAX = QTILE // 128
    DV = NH * 128          # v/proj-shard width
    NG = max(1, DV // 512)  # v-projection head groups (512 cols each)
    GW = DV // NG           # group width
    GH = NH // NG           # heads per group
    TCH = 512               # xt column chunk / qk psq chunk
    NTC = T // TCH
    inv_sqrt_hd = 1.0 / math.sqrt(HD)

    nc = bass.Bass()
    xt_d = nc.dram_tensor("xt", [128, CB, T], BF16, kind="ExternalInput")
    wqk_d = nc.dram_tensor("wqk", [2, NH, 128, C], BF16, kind="ExternalInput")
    wv_d = nc.dram_tensor("wv", [128, CB, DV], BF16, kind="ExternalInput")
    wp_d = nc.dram_tensor("wp", [NH, 128, C], BF16, kind="ExternalInput")
    cos2_d = nc.dram_tensor("cos2", [128, T], BF16, kind="ExternalInput")
    sin2s_d = nc.dram_tensor("sin2s", [128, T], BF16, kind="ExternalInput")
    mask_d = nc.dram_tensor("maskbig", [128, 2 * QTILE - 128], BF16, kind="ExternalInput")
    ones_d = nc.dram_tensor("ones128", [128, 128], BF16, kind="ExternalInput")
    if use_bqkv:
        # [128, 2*NH] per-partition q/k bias columns; V bias via rank-1 matmul
        bqk_d = nc.dram_tensor("bqk", [128, 2 * NH], F32, kind="ExternalInput")
        onecol_d = nc.dram_tensor("onecol", [1, 128], BF16, kind="ExternalInput")
        bv_d = nc.dram_tensor("bv", [1, DV], BF16, kind="ExternalInput")
    out_d = nc.dram_tensor("out_partial", [T, C], BF16, kind="ExternalOutput")

    with tile.TileContext(nc) as tc:
        cms = {}

        def openpool(name, **kw):
            cm = tc.tile_pool(name=name, bufs=1, **kw)
            cms[name] = cm
            return cm.__enter__()

        def close(*names):
            for n in names:
                cms.pop(n).__exit__(None, None, None)

        # ---- pools + tiles up front, in per-side stack order.
        # left SBUF stack (live to the end): cpool..ropool;
        # right stack: xpool, later replaced by wppool.
        cpool = openpool("cpool")
        cos2 = cpool.tile([128, T], BF16, name="cos2")
        sin2s = cpool.tile([128, T], BF16, name="sin2s")
        maskt = cpool.tile([128, 2 * QTILE - 128], BF16, name="maskt")
        ones = cpool.tile([128, 128], BF16, name="ones")
        if use_bqkv:
            bqk = cpool.tile([128, 2 * NH], F32, name="bqk")
            onecol = cpool.tile([1, 128], BF16, name="onecol")
            bv = cpool.tile([1, DV], BF16, name="bv")

        outc_r = [cpool.tile([128, 512], BF16, name=f"outc{i}", tag=f"outc{i}")
                  for i in range(2)]

        ohpool = openpool("ohpool")
        ohs = [ohpool.tile([128, T], BF16, name=f"oh{h}", tag=f"oh{h}")
               for h in range(NH)]

        wqpool = openpool("wqpool")
        wq_r = [(wqpool.tile([128, C], BF16, name=f"wq{i}", tag=f"wq{i}"),
                 wqpool.tile([128, C], BF16, name=f"wk{i}", tag=f"wk{i}"))
                for i in range(2)]

        vpool = openpool("vpool")
        vgs = [[vpool.tile([128, GW], BF16, name=f"vg{g}_{tb}", tag=f"vg{g}_{tb}")
                for tb in range(TBn)] for g in range(NG)]

        qkpool = openpool("qkpool")
        qk_r = [(qkpool.tile([128, T], BF16, name=f"qr{i}", tag=f"qr{i}"),
                 qkpool.tile([128, T], BF16, name=f"kr{i}", tag=f"kr{i}"))
                for i in range(3)]

        ppool = openpool("ppool")
        qb_r = [ppool.tile([128, TCH], BF16, name=f"qb{i}", tag=f"qb{i}")
                for i in range(3)]
        qrot_r = [ppool.tile([128, TCH], BF16, name=f"qrot{i}", tag=f"qrot{i}")
                  for i in range(3)]
        pt_r = [ppool.tile([128, QTILE], BF16, name=f"pt{i}", tag=f"pt{i}")
                for i in range(6)]

        ropool = openpool("ropool")
        rec_r = [ropool.tile([128, QTILE], F32, name="rec0", tag="rec0")]
        racc_r = [ropool.tile([128, QTILE], BF16, name=f"racc{i}", tag=f"racc{i}")
                  for i in range(2)]


        wvpool = openpool("wvpool", side="right")
        wvg_t = wvpool.tile([128, CB, GW], BF16, name="wvg")

        xpool = openpool("xpool", side="right")
        xbig = xpool.tile([128, CB, T], BF16, name="xbig")
        xts = [xbig[:, cb, :] for cb in range(CB)]

        # PSUM: prologue uses psq(2)+psv(2); psv then closes and the
        # attention pools take its banks -> psq2+psS2+psO2+psR2 = 8 banks.
        psqp = openpool("psq", space="PSUM")
        psq_r = [psqp.tile([128, TCH], F32, name=f"psq{i}", tag=f"psq{i}")
                 for i in range(2)]
        psvp = openpool("psv", space="PSUM")
        psv_r = [psvp.tile([128, GW], F32, name=f"psv{i}", tag=f"psv{i}")
                 for i in range(2)]
        psS_r, psO_r = [], []

        ctr = {"psv": 0, "psq": 0, "psS": 0, "psO": 0, "qb": 0, "pt": 0,
               "rec": 0, "racc": 0, "psPsm": 0, "outc": 0}

        def ring(rs, key):
            t = rs[ctr[key] % len(rs)]
            ctr[key] += 1
            return t

        # ---------------- DMA preloads (issue order = queue order) ----------
        def load_wq(h):
            wq, wk = wq_r[h % 2]
            nc.sync.dma_start(out=wq[:], in_=wqk_d[0, h])
            nc.sync.dma_start(out=wk[:], in_=wqk_d[1, h])

        def load_wvg(g):
            h2 = CB // 2
            nc.sync.dma_start(out=wvg_t[:, 0:h2, :],
                              in_=wv_d[:, 0:h2, g * GW:(g + 1) * GW])
            nc.sync.dma_start(out=wvg_t[:, h2:CB, :],
                              in_=wv_d[:, h2:CB, g * GW:(g + 1) * GW])

        def load_xt_chunk(tc_, fine=False):
            if fine:
                # per-cb pieces: compute can start as each lands
                for cb in range(CB):
                    nc.sync.dma_start(out=xbig[:, cb, tc_ * TCH:(tc_ + 1) * TCH],
                                      in_=xt_d[:, cb, tc_ * TCH:(tc_ + 1) * TCH])
            else:
                nc.sync.dma_start(out=xbig[:, :, tc_ * TCH:(tc_ + 1) * TCH],
                                  in_=xt_d[:, :, tc_ * TCH:(tc_ + 1) * TCH])

        nc.sync.dma_start(out=wq_r[0][0][:], in_=wqk_d[0, 0])
        load_xt_chunk(0, fine=True)
        nc.sync.dma_start(out=wq_r[0][1][:], in_=wqk_d[1, 0])
        load_wvg(0)
        load_xt_chunk(1)
        nc.sync.dma_start(out=cos2[:], in_=cos2_d[:])
        nc.sync.dma_start(out=sin2s[:], in_=sin2s_d[:])
        for tc_ in range(2, NTC):
            load_xt_chunk(tc_)
        nc.sync.dma_start(out=maskt[:], in_=mask_d[:])
        nc.sync.dma_start(out=ones[:], in_=ones_d[:])
        load_wq(1)
        if use_bqkv:
            nc.sync.dma_start(out=bqk[:], in_=bqk_d[:])
            nc.sync.dma_start(out=onecol[:], in_=onecol_d[:])
            nc.sync.dma_start(out=bv[:], in_=bv_d[:])

        # ---------------- building blocks ----------------
        def vproj_group(g, tb0, tb1):
            """V columns for head group g, token blocks [tb0, tb1)."""
            for tb in range(tb0, tb1):
                psv = ring(psv_r, "psv")
                for cb in range(CB):
                    nc.tensor.matmul(
                        psv[:], xts[cb][:, tb * 128:(tb + 1) * 128], wvg_t[:, cb, :],
                        start=(cb == 0), stop=(cb == CB - 1 and not use_bqkv))
                if use_bqkv:
                    nc.tensor.matmul(psv[:], onecol[:], bv[:, g * GW:(g + 1) * GW],
                                     start=False, stop=True)
                nc.scalar.copy(out=vgs[g][tb][:], in_=psv[:])

        def rope_tail(h, s, tc_, ps, dmae=None):
            """PSUM chunk -> RoPE -> qr/kr slice (ACT+DMA+DVE, no PE work)."""
            dst = qk_r[h % 3][s]
            ts = slice(tc_ * TCH, (tc_ + 1) * TCH)
            qb = ring(qb_r, "qb")
            qrot = qrot_r[(ctr["qb"] - 1) % len(qrot_r)]
            if use_bqkv:
                nc.vector.tensor_scalar(
                    qb[:], ps[:], bqk[:, s * NH + h:s * NH + h + 1], None, ALU.add)
            else:
                nc.scalar.copy(out=qb[:], in_=ps[:])
            # partition-half swap.  During the prologue the SP queue is
            # congested with preloads -> use the idle gpsimd queue; later the
            # gpsimd queue carries the denominator all-reduces -> use SP.
            dmae = dmae or nc.sync
            dmae.dma_start(out=qrot[0:64, :], in_=qb[64:128, :])
            dmae.dma_start(out=qrot[64:128, :], in_=qb[0:64, :])
            nc.vector.tensor_mul(qb[:], qb[:], cos2[:, ts])
            nc.vector.tensor_mul(qrot[:], qrot[:], sin2s[:, ts])
            nc.vector.tensor_add(dst[:, ts], qb[:], qrot[:])

        def qkproj_chunk(h, s, tc_):
            """psq for (head h, q/k s), token chunk tc_, then RoPE.
            (prologue-only path: swaps ride the idle gpsimd queue)"""
            w = wq_r[h % 2][s]
            ts = slice(tc_ * TCH, (tc_ + 1) * TCH)
            ps = ring(psq_r, "psq")
            for cb in range(CB):
                nc.tensor.matmul(ps[:], w[:, cb * 128:(cb + 1) * 128],
                                 xts[cb][:, ts], start=(cb == 0), stop=(cb == CB - 1))
            rope_tail(h, s, tc_, ps, dmae=nc.gpsimd)

        def qkproj_head(h):
            for tc_ in range(NTC):
                qkproj_chunk(h, 0, tc_)
                qkproj_chunk(h, 1, tc_)

        def proj_gen(h):
            """Generator form of qkproj_head: yields after each PE matmul so
            projection work can be woven into an attention stream."""
            for tc_ in range(NTC):
                for s in (0, 1):
                    w = wq_r[h % 2][s]
                    ts = slice(tc_ * TCH, (tc_ + 1) * TCH)
                    ps = ring(psq_r, "psq")
                    for cb in range(CB):
                        nc.tensor.matmul(
                            ps[:], w[:, cb * 128:(cb + 1) * 128], xts[cb][:, ts],
                            start=(cb == 0), stop=(cb == CB - 1))
                        yield
                    rope_tail(h, s, tc_, ps)

        pending = []   # deferred per-qt R matmuls + norms, shared across heads

        def attn_gen(h, defer=True):
            """Generator: one flash-attention head; yields after each key
            block.  S matmuls run LOOKAHEAD blocks ahead of their O
            consumers so the exp result is ready before the PE needs it --
            the PE must run back-to-back to hold its top p-state.

            Softmax denominators never touch the PE: exp tiles accumulate
            via bf16 DVE adds into racc; a per-query-tile gpsimd
            partition_all_reduce produces the denominator, broadcast to all
            partitions, on the otherwise idle Pool engine."""
            LOOKAHEAD = 2
            qr, kr = qk_r[h % 3]
            g, hi = divmod(h, GH)
            vg = vgs[g]

            for qt in range(NQT):
                ntk = (qt + 1) * JMAX
                tqs = slice(qt * QTILE, (qt + 1) * QTILE)
                psO = ring(psO_r, "psO")
                racc = ring(racc_r, "racc")
                pts = {}

                def emit_S(tkb, ntk=ntk, qt=qt, pts=pts, racc=racc):
                    psS = ring(psS_r, "psS")
                    nc.tensor.matmul(
                        psS[:], kr[:, tkb * 128:(tkb + 1) * 128], qr[:, tqs],
                        start=True, stop=True)
                    pt = ring(pt_r, "pt")
                    nc.scalar.activation(pt[:], psS[:], AF.Exp, scale=inv_sqrt_hd)
                    j = tkb - qt * JMAX
                    if j >= 0:
                        m0 = (JMAX - 1 - j) * 128
                        nc.vector.tensor_mul(pt[:], pt[:], maskt[:, m0:m0 + QTILE])
                    pts[tkb] = pt
                    if tkb >= 1:
                        a = pts[0] if tkb == 1 else racc
                        nc.vector.tensor_add(racc[:], a[:], pt[:])

                def emit_O(tkb, ntk=ntk, pts=pts, psO=psO):
                    pt = pts[tkb]
                    nc.tensor.matmul(psO[:], vg[tkb][:, hi * 128:(hi + 1) * 128],
                                     pt[:], start=(tkb == 0), stop=(tkb == ntk - 1))

                for tkb in range(ntk):
                    emit_S(tkb)
                    if tkb >= LOOKAHEAD:
                        emit_O(tkb - LOOKAHEAD)
                    if pending and (fn := pending.pop(0)) is not None:
                        fn()
                    yield qt
                for tkb in range(max(0, ntk - LOOKAHEAD), ntk):
                    emit_O(tkb)
                    if pending and (fn := pending.pop(0)) is not None:
                        fn()
                    yield qt

                def denom_tail(racc=racc, psO=psO, tqs=tqs):
                    # single ones-matmul on the accumulated exp tile gives the
                    # denominator broadcast across partitions (psPsm banks are
                    # free outside the phase-C stream)
                    rP = ring(psPsm_r, "psPsm")
                    nc.tensor.matmul(rP[:, 0:QTILE], ones[:], racc[:],
                                     start=True, stop=True)
                    rec = ring(rec_r, "rec")
                    nc.vector.reciprocal(rec[:], rP[:, 0:QTILE])
                    nc.vector.tensor_mul(ohs[h][:, tqs], psO[:], rec[:])

                if defer:
                    pending.extend([None] * min(4, max(2, ntk // 4)) + [denom_tail])
                else:
                    denom_tail()

        def run_attn(h, filler=None):
            """Emit attention head h, weaving in filler matmuls (avg 1.5 per
            key block, so the filler spans the whole head) to keep the PE fed
            across the exp latency chain."""
            for i, _ in enumerate(attn_gen(h, defer=True)):
                if filler is not None:
                    for _ in range(2):
                        if next(filler, "done") == "done":
                            filler = None
                            break
            if filler is not None:
                for _ in filler:
                    pass

        psPsm_r = []

        def c_gen(wps):
            """Phase C emitter: psP chunk groups of 8 matmuls, then copy+DMA
            of that [128,512] output chunk. Yields its tb before each PE op."""
            for tb in range(TBn):
                rs = slice(tb * 128, (tb + 1) * 128)
                for c0 in range(0, C, 512):
                    psp = ring(psPsm_r, "psPsm")
                    for hd in range(NH):
                        yield tb
                        nc.tensor.matmul(
                            psp[:], ohs[hd][:, rs], wps[hd][:, c0:c0 + 512],
                            start=(hd == 0), stop=(hd == NH - 1))
                    ob = ring(outc_r, "outc")
                    nc.scalar.copy(out=ob[:], in_=psp[:])
                    nc.sync.dma_start(out=out_d[rs, c0:c0 + 512], in_=ob[:])

        def run_attn_pair_with_c(h0, h1, cg):
            """Interleave the last two attention heads block-by-block (each
            absorbs the other's softmax latency) and weave in phase-C chunk
            matmuls for query tiles whose normalization is already done."""
            g0, g1 = attn_gen(h0, defer=False), attn_gen(h1, defer=False)
            q0 = q1 = -1
            c_tb = next(cg)          # tb of the NEXT pending C matmul
            while True:
                step = False
                if q0 is not None:
                    q0 = next(g0, None)
                    step = step or q0 is not None
                if q1 is not None:
                    q1 = next(g1, None)
                    step = step or q1 is not None
                if not step:
                    break
                # pair norms are inline: qt i fully normed once both
                # generators are past it
                qmin = min(q0 if q0 is not None else NQT,
                           q1 if q1 is not None else NQT)
                for _ in range(4):
                    if c_tb is None or c_tb // JMAX + 1 > qmin:
                        break
                    c_tb = next(cg, None)
            # flush deferred norms, then drain the rest of phase C
            for fn in pending:
                if fn is not None:
                    fn()
            pending.clear()
            for _ in cg:
                pass

        # ---------------- prologue: v (all groups) + heads 0,1 projections --
        tbpg = TBn // NTC
        for tc_ in range(NTC):
            qkproj_chunk(0, 0, tc_)
            qkproj_chunk(0, 1, tc_)
            vproj_group(0, tc_ * tbpg, (tc_ + 1) * tbpg)
        if NH > 2:
            load_wq(2)
        if NG > 1:
            load_wvg(1)
            for tc_ in range(NTC):
                qkproj_chunk(1, 0, tc_)
                qkproj_chunk(1, 1, tc_)
                vproj_group(1, tc_ * tbpg, (tc_ + 1) * tbpg)
        else:
            qkproj_head(1)

        # swap psv banks for the attention accumulators + small phase-C psP:
        # psq(2) + psS(2) + psO(2) + psPsm(2) = 8 banks, static to the end
        close("psv")
        psSp = openpool("psS", space="PSUM")
        psS_r.extend(psSp.tile([128, QTILE], F32, name=f"psS{i}", tag=f"psS{i}")
                     for i in range(2))
        psOp = openpool("psO", space="PSUM")
        psO_r.extend(psOp.tile([128, QTILE], F32, name=f"psO{i}", tag=f"psO{i}")
                     for i in range(2))
        psPp2 = openpool("psPsm", space="PSUM")
        psPsm_r.extend(psPp2.tile([128, 512], F32, name=f"psPsm{i}", tag=f"psPsm{i}")
                       for i in range(2))

        # ---- steady state: attn(h) with head h+2's projection woven in ----
        for h in range(max(0, NH - 2)):
            if h + 3 < NH:
                load_wq(h + 3)
            run_attn(h, proj_gen(h + 2))

        # x is done (last read: head NH-1's projection); its SBUF hosts the
        # c_proj weights, whose DMA overlaps the last two attention heads
        close("xpool")
        wppool = tc.tile_pool(name="wppool", bufs=1, side="right")
        cms["wppool"] = wppool
        wppool_p = wppool.__enter__()
        wps = []
        for hd in range(NH):
            wpt = wppool_p.tile([128, C], BF16, name=f"wp{hd}", tag=f"wp{hd}")
            nc.sync.dma_start(out=wpt[:], in_=wp_d[hd])
            wps.append(wpt)

        # last heads: serial (block-interleaving two attention streams
        # miscompares on the hw path); phase C weaves into head NH-1 only,
        # gated on its query-tile progress (all other heads are done)
        run_attn(NH - 2)
        cg = c_gen(wps)
        c_tb = next(cg)
        for q in attn_gen(NH - 1, defer=False):
            for _ in range(4):
                if c_tb is None or c_tb // JMAX + 1 > q:
                    break
                c_tb = next(cg, None)
        for fn in pending:
            if fn is not None:
                fn()
        pending.clear()
        for _ in cg:
            pass

        close("psPsm", "psO", "psS", "psq",
              "wppool", "wvpool",
              "ropool", "ppool", "qkpool", "vpool",
              "wqpool", "ohpool", "cpool")

    if legalize:
        _legalize_waits(nc)
    return nc


# ---------------------------------------------------------------- host side

_PERM = np.concatenate([np.arange(0, HD, 2), np.arange(1, HD, 2)])  # de-interleave


def shard_core(core, x, freqs_cos, freqs_sin, Wqkv, bqkv, Wproj,
               T=T, C=C, NH=NH, qtile=256, use_bqkv=False):
    """Build the in_map for one core."""
    CB = C // 128
    DV = NH * 128
    QTILE = min(qtile, T)
    b = core // 2
    hb = (core % 2) * NH

    xt = np.ascontiguousarray(
        x[b].T.reshape(CB, 128, T).transpose(1, 0, 2)).astype(NPBF)

    # [2, NH, 128] column indices (q/k, de-interleaved within each head)
    cols = (np.arange(2)[:, None, None] * C
            + (hb + np.arange(NH))[None, :, None] * HD + _PERM[None, None, :])
    wqk = Wqkv[:, cols]                              # [C, 2, NH, 128]
    wqk = np.ascontiguousarray(
        wqk.reshape(CB, 128, 2, NH, 128).transpose(2, 3, 1, 0, 4)
        .reshape(2, NH, 128, C)).astype(NPBF)

    wv = np.ascontiguousarray(
        Wqkv[:, 2 * C + hb * HD: 2 * C + (hb + NH) * HD]
        .reshape(CB, 128, DV).transpose(1, 0, 2)).astype(NPBF)
    wp = np.ascontiguousarray(
        Wproj[hb * HD:(hb + NH) * HD, :].reshape(NH, 128, C)).astype(NPBF)

    cos2 = np.concatenate([freqs_cos.T, freqs_cos.T], 0)
    cos2 = np.ascontiguousarray(cos2).astype(NPBF)   # [128, T]
    sin2s = np.concatenate([-freqs_sin.T, freqs_sin.T], 0)
    sin2s = np.ascontiguousarray(sin2s).astype(NPBF)

    u = np.arange(2 * QTILE - 128)[None, :]
    p = np.arange(128)[:, None]
    maskbig = (p <= u - (QTILE - 128)).astype(NPBF)

    im = {
        "xt": xt, "wqk": wqk, "wv": wv, "wp": wp,
        "cos2": cos2, "sin2s": sin2s, "maskbig": maskbig,
        "ones128": np.ones((128, 128), NPBF),
    }
    if use_bqkv:
        bqk = np.empty((128, 2 * NH), np.float32)
        for s in range(2):
            for h in range(NH):
                bqk[:, s * NH + h] = bqkv[s * C + (hb + h) * HD + _PERM]
        im["bqk"] = bqk
        im["onecol"] = np.ones((1, 128), NPBF)
        im["bv"] = np.ascontiguousarray(
            bqkv[2 * C + hb * HD: 2 * C + (hb + NH) * HD][None, :]).astype(NPBF)
    return im


_CACHE = {}


def _get_program(use_bqkv):
    key = use_bqkv
    if key not in _CACHE:
        _CACHE[key] = build_program(use_bqkv=use_bqkv)
    return _CACHE[key]


def kernel(x, freqs_cos, freqs_sin, Wqkv, bqkv, Wproj, bproj):
    x = np.asarray(x, np.float32)
    freqs_cos = np.asarray(freqs_cos, np.float32)
    freqs_sin = np.asarray(freqs_sin, np.float32)
    Wqkv = np.asarray(Wqkv, np.float32)
    bqkv = np.asarray(bqkv, np.float32)
    Wproj = np.asarray(Wproj, np.float32)
    bproj = np.asarray(bproj, np.float32)

    use_bqkv = bool(np.any(bqkv != 0))
    nc = _get_program(use_bqkv)
    in_maps = [
        shard_core(c, x, freqs_cos, freqs_sin, Wqkv, bqkv, Wproj,
                   use_bqkv=use_bqkv)
        for c in range(NCORES)
    ]
    try:
        res = run_bass_kernel_spmd(nc, in_maps, list(range(NCORES))).results
    except Exception:
        # transient device faults have been observed; retry once
        res = run_bass_kernel_spmd(nc, in_maps, list(range(NCORES))).results

    out = np.empty((B, T, C), np.float32)
    for b in range(B):
        out[b] = (res[2 * b]["out_partial"].astype(np.float32)
                  + res[2 * b + 1]["out_partial"].astype(np.float32))
    out += bproj[None, None, :]
    return out


# revision 59
# speedup vs baseline: 1.2372x; 1.0002x over previous
"""Causal self-attention (B=4, T=2048, C=2048, H=16, HD=128) on 8 trn2 cores.

Sharding: core c handles batch b = c//2 and heads (c%2)*8 .. +8.
  - QKV projection column-sharded by head, attention head-sharded,
    c_proj row-sharded; the pair partial sums are combined on host.

v3: all matmul operands bf16 (same PE rate as f32r in the cost model,
half the DMA/SBUF), fully SBUF-resident pipeline (no q/k/v DRAM spill),
fused per-head schedule: project head h+1 while attention for head h
runs on the PE; RoPE on DVE overlaps attention; flash tiles of 256
queries to trim the causal diagonal.

Self-contained: hardcodes shapes; builds one SPMD Bass program and runs
it on cores 0-7 via run_bass_kernel_spmd.
"""
import math

import ml_dtypes
import numpy as np

import concourse.bass as bass
import concourse.bass_isa as bass_isa
import concourse.library_config as library_config
import concourse.mybir as mybir
import concourse.tile as tile
from concourse.bass_utils import run_bass_kernel_spmd

F32 = mybir.dt.float32
BF16 = mybir.dt.bfloat16
AF = mybir.ActivationFunctionType
ALU = mybir.AluOpType
NPBF = ml_dtypes.bfloat16

# problem dims
B, T, C, H = 4, 2048, 2048, 16
HD = 128
NCORES = 8
NH = H // 2          # heads per core

_ctr = [0]


def _legalize_waits(nc, max_waits=1):
    """This walrus build rejects >1 sync wait per instruction. Hoist extra
    waits onto same-engine NoOps inserted directly before the instruction."""
    n_split = 0
    for f in nc.m.functions:
        for blk in f.blocks:
            newil = []
            changed = False
            for inst in blk.instructions:
                si = inst.sync_info
                if si is not None and si.on_wait and len(si.on_wait) > max_waits:
                    waits = list(si.on_wait)
                    for w in waits[:-max_waits]:
                        _ctr[0] += 1
                        nop = mybir.InstNoOp(name=f"I-waitfix-{_ctr[0]}")
                        nop.engine = inst.engine
                        nop.sync_info = mybir.SyncInfo(on_wait=[w], on_update=[])
                        newil.append(nop)
                    inst.sync_info = mybir.SyncInfo(
                        on_wait=waits[-max_waits:], on_update=list(si.on_update)
                    )
                    changed = True
                    n_split += 1
                newil.append(inst)
            if changed:
                blk.instructions = newil
    return n_split


def build_program(T=T, C=C, NH=NH, use_bqkv=False, qtile=256, legalize=True):
    """One core's program: full pipeline for (1 batch, NH heads)."""
    CB = C // 128          # contraction blocks
    TBn = T // 128         # token blocks
    QTILE = min(qtile, T)  # flash query-tile
    NQT = T // QTILE
    JMAX = QTILE // 128
    DV = NH * 128          # v/proj-shard width
    NG = max(1, DV // 512)  # v-projection head groups (512 cols each)
    GW = DV // NG           # group width
    GH = NH // NG           # heads per group
    TCH = 512               # xt column chunk / qk psq chunk
    NTC = T // TCH
    inv_sqrt_hd = 1.0 / math.sqrt(HD)

    nc = bass.Bass()
    xt_d = nc.dram_tensor("xt", [128, CB, T], BF16, kind="ExternalInput")
    wqk_d = nc.dram_tensor("wqk", [2, NH, 128, C], BF16, kind="ExternalInput")
    wv_d = nc.dram_tensor("wv", [128, CB, DV], BF16, kind="ExternalInput")
    wp_d = nc.dram_tensor("wp", [NH, 128, C], BF16, kind="ExternalInput")
    cos2_d = nc.dram_tensor("cos2", [128, T], BF16, kind="ExternalInput")
    sin2s_d = nc.dram_tensor("sin2s", [128, T], BF16, kind="ExternalInput")
    mask_d = nc.dram_tensor("maskbig", [128, 2 * QTILE - 128], BF16, kind="ExternalInput")
    ones_d = nc.dram_tensor("ones128", [128, 128], BF16, kind="ExternalInput")
    if use_bqkv:
        # [128, 2*NH] per-partition q/k bias columns; V bias via rank-1 matmul
        bqk_d = nc.dram_tensor("bqk", [128, 2 * NH], F32, kind="ExternalInput")
        onecol_d = nc.dram_tensor("onecol", [1, 128], BF16, kind="ExternalInput")
        bv_d = nc.dram_tensor("bv", [1, DV], BF16, kind="ExternalInput")
    out_d = nc.dram_tensor("out_partial", [T, C], BF16, kind="ExternalOutput")

    with tile.TileContext(nc) as tc:
        cms = {}

        def openpool(name, **kw):
            cm = tc.tile_pool(name=name, bufs=1, **kw)
            cms[name] = cm
            return cm.__enter__()

        def close(*names):
            for n in names:
                cms.pop(n).__exit__(None, None, None)

        # ---- pools + tiles up front, in per-side stack order.
        # left SBUF stack (live to the end): cpool..ropool;
        # right stack: xpool, later replaced by wppool.
        cpool = openpool("cpool")
        cos2 = cpool.tile([128, T], BF16, name="cos2")
        sin2s = cpool.tile([128, T], BF16, name="sin2s")
        maskt = cpool.tile([128, 2 * QTILE - 128], BF16, name="maskt")
        ones = cpool.tile([128, 128], BF16, name="ones")
        if use_bqkv:
            bqk = cpool.tile([128, 2 * NH], F32, name="bqk")
            onecol = cpool.tile([1, 128], BF16, name="onecol")
            bv = cpool.tile([1, DV], BF16, name="bv")

        outc_r = [cpool.tile([128, 512], BF16, name=f"outc{i}", tag=f"outc{i}")
                  for i in range(2)]

        ohpool = openpool("ohpool")
        ohs = [ohpool.tile([128, T], BF16, name=f"oh{h}", tag=f"oh{h}")
               for h in range(NH)]

        wqpool = openpool("wqpool")
        wq_r = [(wqpool.tile([128, C], BF16, name=f"wq{i}", tag=f"wq{i}"),
                 wqpool.tile([128, C], BF16, name=f"wk{i}", tag=f"wk{i}"))
                for i in range(2)]

        vpool = openpool("vpool")
        vgs = [[vpool.tile([128, GW], BF16, name=f"vg{g}_{tb}", tag=f"vg{g}_{tb}")
                for tb in range(TBn)] for g in range(NG)]

        qkpool = openpool("qkpool")
        qk_r = [(qkpool.tile([128, T], BF16, name=f"qr{i}", tag=f"qr{i}"),
                 qkpool.tile([128, T], BF16, name=f"kr{i}", tag=f"kr{i}"))
                for i in range(3)]

        ppool = openpool("ppool")
        qb_r = [ppool.tile([128, TCH], BF16, name=f"qb{i}", tag=f"qb{i}")
                for i in range(3)]
        qrot_r = [ppool.tile([128, TCH], BF16, name=f"qrot{i}", tag=f"qrot{i}")
                  for i in range(3)]
        pt_r = [ppool.tile([128, QTILE], BF16, name=f"pt{i}", tag=f"pt{i}")
                for i in range(6)]

        ropool = openpool("ropool")
        rec_r = [ropool.tile([128, QTILE], F32, name="rec0", tag="rec0")]
        racc_r = [ropool.tile([128, QTILE], BF16, name=f"racc{i}", tag=f"racc{i}")
                  for i in range(2)]


        wvpool = openpool("wvpool", side="right")
        wvg_t = wvpool.tile([128, CB, GW], BF16, name="wvg")

        xpool = openpool("xpool", side="right")
        xbig = xpool.tile([128, CB, T], BF16, name="xbig")
        xts = [xbig[:, cb, :] for cb in range(CB)]

        # PSUM: prologue uses psq(2)+psv(2); psv then closes and the
        # attention pools take its banks -> psq2+psS2+psO2+psR2 = 8 banks.
        psqp = openpool("psq", space="PSUM")
        psq_r = [psqp.tile([128, TCH], F32, name=f"psq{i}", tag=f"psq{i}")
                 for i in range(2)]
        psvp = openpool("psv", space="PSUM")
        psv_r = [psvp.tile([128, GW], F32, name=f"psv{i}", tag=f"psv{i}")
                 for i in range(2)]
        psS_r, psO_r = [], []

        ctr = {"psv": 0, "psq": 0, "psS": 0, "psO": 0, "qb": 0, "pt": 0,
               "rec": 0, "racc": 0, "psPsm": 0, "outc": 0}

        def ring(rs, key):
            t = rs[ctr[key] % len(rs)]
            ctr[key] += 1
            return t

        # ---------------- DMA preloads (issue order = queue order) ----------
        def load_wq(h):
            wq, wk = wq_r[h % 2]
            nc.sync.dma_start(out=wq[:], in_=wqk_d[0, h])
            nc.sync.dma_start(out=wk[:], in_=wqk_d[1, h])

        def load_wvg(g):
            h2 = CB // 2
            nc.sync.dma_start(out=wvg_t[:, 0:h2, :],
                              in_=wv_d[:, 0:h2, g * GW:(g + 1) * GW])
            nc.sync.dma_start(out=wvg_t[:, h2:CB, :],
                              in_=wv_d[:, h2:CB, g * GW:(g + 1) * GW])

        def load_xt_chunk(tc_, fine=False):
            if fine:
                # per-cb pieces: compute can start as each lands
                for cb in range(CB):
                    nc.sync.dma_start(out=xbig[:, cb, tc_ * TCH:(tc_ + 1) * TCH],
                                      in_=xt_d[:, cb, tc_ * TCH:(tc_ + 1) * TCH])
            else:
                nc.sync.dma_start(out=xbig[:, :, tc_ * TCH:(tc_ + 1) * TCH],
                                  in_=xt_d[:, :, tc_ * TCH:(tc_ + 1) * TCH])

        nc.sync.dma_start(out=wq_r[0][0][:], in_=wqk_d[0, 0])
        load_xt_chunk(0, fine=True)
        nc.sync.dma_start(out=wq_r[0][1][:], in_=wqk_d[1, 0])
        load_wvg(0)
        load_xt_chunk(1, fine=True)
        nc.sync.dma_start(out=cos2[:], in_=cos2_d[:])
        nc.sync.dma_start(out=sin2s[:], in_=sin2s_d[:])
        for tc_ in range(2, NTC):
            load_xt_chunk(tc_)
        nc.sync.dma_start(out=maskt[:], in_=mask_d[:])
        nc.sync.dma_start(out=ones[:], in_=ones_d[:])
        load_wq(1)
        if use_bqkv:
            nc.sync.dma_start(out=bqk[:], in_=bqk_d[:])
            nc.sync.dma_start(out=onecol[:], in_=onecol_d[:])
            nc.sync.dma_start(out=bv[:], in_=bv_d[:])

        # ---------------- building blocks ----------------
        def vproj_group(g, tb0, tb1):
            """V columns for head group g, token blocks [tb0, tb1)."""
            for tb in range(tb0, tb1):
                psv = ring(psv_r, "psv")
                for cb in range(CB):
                    nc.tensor.matmul(
                        psv[:], xts[cb][:, tb * 128:(tb + 1) * 128], wvg_t[:, cb, :],
                        start=(cb == 0), stop=(cb == CB - 1 and not use_bqkv))
                if use_bqkv:
                    nc.tensor.matmul(psv[:], onecol[:], bv[:, g * GW:(g + 1) * GW],
                                     start=False, stop=True)
                nc.scalar.copy(out=vgs[g][tb][:], in_=psv[:])

        def rope_tail(h, s, tc_, ps, dmae=None):
            """PSUM chunk -> RoPE -> qr/kr slice (ACT+DMA+DVE, no PE work)."""
            dst = qk_r[h % 3][s]
            ts = slice(tc_ * TCH, (tc_ + 1) * TCH)
            qb = ring(qb_r, "qb")
            qrot = qrot_r[(ctr["qb"] - 1) % len(qrot_r)]
            if use_bqkv:
                nc.vector.tensor_scalar(
                    qb[:], ps[:], bqk[:, s * NH + h:s * NH + h + 1], None, ALU.add)
            else:
                nc.scalar.copy(out=qb[:], in_=ps[:])
            # partition-half swap.  During the prologue the SP queue is
            # congested with preloads -> use the idle gpsimd queue; later the
            # gpsimd queue carries the denominator all-reduces -> use SP.
            dmae = dmae or nc.sync
            dmae.dma_start(out=qrot[0:64, :], in_=qb[64:128, :])
            dmae.dma_start(out=qrot[64:128, :], in_=qb[0:64, :])
            nc.vector.tensor_mul(qb[:], qb[:], cos2[:, ts])
            nc.vector.tensor_mul(qrot[:], qrot[:], sin2s[:, ts])
            nc.vector.tensor_add(dst[:, ts], qb[:], qrot[:])

        def qkproj_chunk(h, s, tc_):
            """psq for (head h, q/k s), token chunk tc_, then RoPE.
            (prologue-only path: swaps ride the idle gpsimd queue)"""
            w = wq_r[h % 2][s]
            ts = slice(tc_ * TCH, (tc_ + 1) * TCH)
            ps = ring(psq_r, "psq")
            for cb in range(CB):
                nc.tensor.matmul(ps[:], w[:, cb * 128:(cb + 1) * 128],
                                 xts[cb][:, ts], start=(cb == 0), stop=(cb == CB - 1))
            rope_tail(h, s, tc_, ps, dmae=nc.gpsimd)

        def qkproj_head(h):
            for tc_ in range(NTC):
                qkproj_chunk(h, 0, tc_)
                qkproj_chunk(h, 1, tc_)

        def proj_gen(h):
            """Generator form of qkproj_head: yields after each PE matmul so
            projection work can be woven into an attention stream."""
            for tc_ in range(NTC):
                for s in (0, 1):
                    w = wq_r[h % 2][s]
                    ts = slice(tc_ * TCH, (tc_ + 1) * TCH)
                    ps = ring(psq_r, "psq")
                    for cb in range(CB):
                        nc.tensor.matmul(
                            ps[:], w[:, cb * 128:(cb + 1) * 128], xts[cb][:, ts],
                            start=(cb == 0), stop=(cb == CB - 1))
                        yield
                    rope_tail(h, s, tc_, ps)

        pending = []   # deferred per-qt R matmuls + norms, shared across heads

        def attn_gen(h, defer=True):
            """Generator: one flash-attention head; yields after each key
            block.  S matmuls run LOOKAHEAD blocks ahead of their O
            consumers so the exp result is ready before the PE needs it --
            the PE must run back-to-back to hold its top p-state.

            Softmax denominators never touch the PE: exp tiles accumulate
            via bf16 DVE adds into racc; a per-query-tile gpsimd
            partition_all_reduce produces the denominator, broadcast to all
            partitions, on the otherwise idle Pool engine."""
            LOOKAHEAD = 2
            qr, kr = qk_r[h % 3]
            g, hi = divmod(h, GH)
            vg = vgs[g]

            for qt in range(NQT):
                ntk = (qt + 1) * JMAX
                tqs = slice(qt * QTILE, (qt + 1) * QTILE)
                psO = ring(psO_r, "psO")
                racc = ring(racc_r, "racc")
                pts = {}

                def emit_S(tkb, ntk=ntk, qt=qt, pts=pts, racc=racc):
                    psS = ring(psS_r, "psS")
                    nc.tensor.matmul(
                        psS[:], kr[:, tkb * 128:(tkb + 1) * 128], qr[:, tqs],
                        start=True, stop=True)
                    pt = ring(pt_r, "pt")
                    nc.scalar.activation(pt[:], psS[:], AF.Exp, scale=inv_sqrt_hd)
                    j = tkb - qt * JMAX
                    if j >= 0:
                        m0 = (JMAX - 1 - j) * 128
                        nc.vector.tensor_mul(pt[:], pt[:], maskt[:, m0:m0 + QTILE])
                    pts[tkb] = pt
                    if tkb >= 1:
                        a = pts[0] if tkb == 1 else racc
                        nc.vector.tensor_add(racc[:], a[:], pt[:])

                def emit_O(tkb, ntk=ntk, pts=pts, psO=psO):
                    pt = pts[tkb]
                    nc.tensor.matmul(psO[:], vg[tkb][:, hi * 128:(hi + 1) * 128],
                                     pt[:], start=(tkb == 0), stop=(tkb == ntk - 1))

                for tkb in range(ntk):
                    emit_S(tkb)
                    if tkb >= LOOKAHEAD:
                        emit_O(tkb - LOOKAHEAD)
                    if pending and (fn := pending.pop(0)) is not None:
                        fn()
                    yield qt
                for tkb in range(max(0, ntk - LOOKAHEAD), ntk):
                    emit_O(tkb)
                    if pending and (fn := pending.pop(0)) is not None:
                        fn()
                    yield qt

                def denom_tail(racc=racc, psO=psO, tqs=tqs):
                    # single ones-matmul on the accumulated exp tile gives the
                    # denominator broadcast across partitions (psPsm banks are
                    # free outside the phase-C stream)
                    rP = ring(psPsm_r, "psPsm")
                    nc.tensor.matmul(rP[:, 0:QTILE], ones[:], racc[:],
                                     start=True, stop=True)
                    rec = ring(rec_r, "rec")
                    nc.vector.reciprocal(rec[:], rP[:, 0:QTILE])
                    nc.vector.tensor_mul(ohs[h][:, tqs], psO[:], rec[:])

                if defer:
                    pending.extend([None] * min(4, max(2, ntk // 4)) + [denom_tail])
                else:
                    denom_tail()

        def run_attn(h, filler=None):
            """Emit attention head h, weaving in filler matmuls (avg 1.5 per
            key block, so the filler spans the whole head) to keep the PE fed
            across the exp latency chain."""
            for i, _ in enumerate(attn_gen(h, defer=True)):
                if filler is not None:
                    for _ in range(2):
                        if next(filler, "done") == "done":
                            filler = None
                            break
            if filler is not None:
                for _ in filler:
                    pass

        psPsm_r = []

        def c_gen(wps):
            """Phase C emitter: psP chunk groups of 8 matmuls, then copy+DMA
            of that [128,512] output chunk. Yields its tb before each PE op."""
            for tb in range(TBn):
                rs = slice(tb * 128, (tb + 1) * 128)
                for c0 in range(0, C, 512):
                    psp = ring(psPsm_r, "psPsm")
                    for hd in range(NH):
                        yield tb
                        nc.tensor.matmul(
                            psp[:], ohs[hd][:, rs], wps[hd][:, c0:c0 + 512],
                            start=(hd == 0), stop=(hd == NH - 1))
                    ob = ring(outc_r, "outc")
                    nc.scalar.copy(out=ob[:], in_=psp[:])
                    nc.sync.dma_start(out=out_d[rs, c0:c0 + 512], in_=ob[:])

        def run_attn_pair_with_c(h0, h1, cg):
            """Interleave the last two attention heads block-by-block (each
            absorbs the other's softmax latency) and weave in phase-C chunk
            matmuls for query tiles whose normalization is already done."""
            g0, g1 = attn_gen(h0, defer=False), attn_gen(h1, defer=False)
            q0 = q1 = -1
            c_tb = next(cg)          # tb of the NEXT pending C matmul
            while True:
                step = False
                if q0 is not None:
                    q0 = next(g0, None)
                    step = step or q0 is not None
                if q1 is not None:
                    q1 = next(g1, None)
                    step = step or q1 is not None
                if not step:
                    break
                # pair norms are inline: qt i fully normed once both
                # generators are past it
                qmin = min(q0 if q0 is not None else NQT,
                           q1 if q1 is not None else NQT)
                for _ in range(4):
                    if c_tb is None or c_tb // JMAX + 1 > qmin:
                        break
                    c_tb = next(cg, None)
            # flush deferred norms, then drain the rest of phase C
            for fn in pending:
                if fn is not None:
                    fn()
            pending.clear()
            for _ in cg:
                pass

        # ---------------- prologue: v (all groups) + heads 0,1 projections --
        tbpg = TBn // NTC
        for tc_ in range(NTC):
            qkproj_chunk(0, 0, tc_)
            qkproj_chunk(0, 1, tc_)
            vproj_group(0, tc_ * tbpg, (tc_ + 1) * tbpg)
        if NH > 2:
            load_wq(2)
        if NG > 1:
            load_wvg(1)
            for tc_ in range(NTC):
                qkproj_chunk(1, 0, tc_)
                qkproj_chunk(1, 1, tc_)
                vproj_group(1, tc_ * tbpg, (tc_ + 1) * tbpg)
        else:
            qkproj_head(1)

        # swap psv banks for the attention accumulators + small phase-C psP:
        # psq(2) + psS(2) + psO(2) + psPsm(2) = 8 banks, static to the end
        close("psv")
        psSp = openpool("psS", space="PSUM")
        psS_r.extend(psSp.tile([128, QTILE], F32, name=f"psS{i}", tag=f"psS{i}")
                     for i in range(2))
        psOp = openpool("psO", space="PSUM")
        psO_r.extend(psOp.tile([128, QTILE], F32, name=f"psO{i}", tag=f"psO{i}")
                     for i in range(2))
        psPp2 = openpool("psPsm", space="PSUM")
        psPsm_r.extend(psPp2.tile([128, 512], F32, name=f"psPsm{i}", tag=f"psPsm{i}")
                       for i in range(2))

        # ---- steady state: attn(h) with head h+2's projection woven in ----
        for h in range(max(0, NH - 2)):
            if h + 3 < NH:
                load_wq(h + 3)
            run_attn(h, proj_gen(h + 2))

        # x is done (last read: head NH-1's projection); its SBUF hosts the
        # c_proj weights, whose DMA overlaps the last two attention heads
        close("xpool")
        wppool = tc.tile_pool(name="wppool", bufs=1, side="right")
        cms["wppool"] = wppool
        wppool_p = wppool.__enter__()
        wps = []
        for hd in range(NH):
            wpt = wppool_p.tile([128, C], BF16, name=f"wp{hd}", tag=f"wp{hd}")
            nc.sync.dma_start(out=wpt[:], in_=wp_d[hd])
            wps.append(wpt)

        # last heads: serial (block-interleaving two attention streams
        # miscompares on the hw path); phase C weaves into head NH-1 only,
        # gated on its query-tile progress (all other heads are done)
        run_attn(NH - 2)
        cg = c_gen(wps)
        c_tb = next(cg)
        for q in attn_gen(NH - 1, defer=False):
            for _ in range(4):
                if c_tb is None or c_tb // JMAX + 1 > q:
                    break
                c_tb = next(cg, None)
        for fn in pending:
            if fn is not None:
                fn()
        pending.clear()
        for _ in cg:
            pass

        close("psPsm", "psO", "psS", "psq",
              "wppool", "wvpool",
              "ropool", "ppool", "qkpool", "vpool",
              "wqpool", "ohpool", "cpool")

    if legalize:
        _legalize_waits(nc)
    return nc


# ---------------------------------------------------------------- host side

_PERM = np.concatenate([np.arange(0, HD, 2), np.arange(1, HD, 2)])  # de-interleave


def shard_core(core, x, freqs_cos, freqs_sin, Wqkv, bqkv, Wproj,
               T=T, C=C, NH=NH, qtile=256, use_bqkv=False):
    """Build the in_map for one core."""
    CB = C // 128
    DV = NH * 128
    QTILE = min(qtile, T)
    b = core // 2
    hb = (core % 2) * NH

    xt = np.ascontiguousarray(
        x[b].T.reshape(CB, 128, T).transpose(1, 0, 2)).astype(NPBF)

    # [2, NH, 128] column indices (q/k, de-interleaved within each head)
    cols = (np.arange(2)[:, None, None] * C
            + (hb + np.arange(NH))[None, :, None] * HD + _PERM[None, None, :])
    wqk = Wqkv[:, cols]                              # [C, 2, NH, 128]
    wqk = np.ascontiguousarray(
        wqk.reshape(CB, 128, 2, NH, 128).transpose(2, 3, 1, 0, 4)
        .reshape(2, NH, 128, C)).astype(NPBF)

    wv = np.ascontiguousarray(
        Wqkv[:, 2 * C + hb * HD: 2 * C + (hb + NH) * HD]
        .reshape(CB, 128, DV).transpose(1, 0, 2)).astype(NPBF)
    wp = np.ascontiguousarray(
        Wproj[hb * HD:(hb + NH) * HD, :].reshape(NH, 128, C)).astype(NPBF)

    cos2 = np.concatenate([freqs_cos.T, freqs_cos.T], 0)
    cos2 = np.ascontiguousarray(cos2).astype(NPBF)   # [128, T]
    sin2s = np.concatenate([-freqs_sin.T, freqs_sin.T], 0)
    sin2s = np.ascontiguousarray(sin2s).astype(NPBF)

    u = np.arange(2 * QTILE - 128)[None, :]
    p = np.arange(128)[:, None]
    maskbig = (p <= u - (QTILE - 128)).astype(NPBF)

    im = {
        "xt": xt, "wqk": wqk, "wv": wv, "wp": wp,
        "cos2": cos2, "sin2s": sin2s, "maskbig": maskbig,
        "ones128": np.ones((128, 128), NPBF),
    }
    if use_bqkv:
        bqk = np.empty((128, 2 * NH), np.float32)
        for s in range(2):
            for h in range(NH):
                bqk[:, s * NH + h] = bqkv[s * C + (hb + h) * HD + _PERM]
        im["bqk"] = bqk
        im["onecol"] = np.ones((1, 128), NPBF)
        im["bv"] = np.ascontiguousarray(
            bqkv[2 * C + hb * HD: 2 * C + (hb + NH) * HD][None, :]).astype(NPBF)
    return im


_CACHE = {}


def _get_program(use_bqkv):
    key = use_bqkv
    if key not in _CACHE:
        _CACHE[key] = build_program(use_bqkv=use_bqkv)
    return _CACHE[key]


def kernel(x, freqs_cos, freqs_sin, Wqkv, bqkv, Wproj, bproj):
    x = np.asarray(x, np.float32)
    freqs_cos = np.asarray(freqs_cos, np.float32)
    freqs_sin = np.asarray(freqs_sin, np.float32)
    Wqkv = np.asarray(Wqkv, np.float32)
    bqkv = np.asarray(bqkv, np.float32)
    Wproj = np.asarray(Wproj, np.float32)
    bproj = np.asarray(bproj, np.float32)

    use_bqkv = bool(np.any(bqkv != 0))
    nc = _get_program(use_bqkv)
    in_maps = [
        shard_core(c, x, freqs_cos, freqs_sin, Wqkv, bqkv, Wproj,
                   use_bqkv=use_bqkv)
        for c in range(NCORES)
    ]
    try:
        res = run_bass_kernel_spmd(nc, in_maps, list(range(NCORES))).results
    except Exception:
        # transient device faults have been observed; retry once
        res = run_bass_kernel_spmd(nc, in_maps, list(range(NCORES))).results

    out = np.empty((B, T, C), np.float32)
    for b in range(B):
        out[b] = (res[2 * b]["out_partial"].astype(np.float32)
                  + res[2 * b + 1]["out_partial"].astype(np.float32))
    out += bproj[None, None, :]
    return out
